# revision 1
# baseline (speedup 1.0000x reference)
"""Trainium2 Bass kernel for nn_AttentionWithVQ (B=4, N=2048, DIM=512, H=8,
depthwise-conv positional term, softmax attention, output projection).

Sharding: data-parallel over B (4 batches x 2 core-groups) and tensor-parallel
over heads (4 heads per core) -> 8 cores, fully independent per core except a
final partial-sum over the two head-groups of each batch, done on host at
gather time (the output projection contracts over heads).

Core algorithmic fusion: the score matrix
    S = 0.5*(scale * q @ k^T + scale * conv1(m) @ conv2(s)^T)
is ONE matmul over a concatenated 128-feature axis:
    S = Qp^T @ Kp,  Qp = [q*scale*0.5 ; conv1(m)*scale*0.5], Kp = [k ; conv2(s)]
which exactly fills the 128x128 PE array contraction dim.

Softmax denominators come for free by appending a ones-column to V
(attn@[V|1] yields the row-sums of exp(S) in the last output row); exp() is
numerically safe without max-subtraction for this problem's score magnitudes
(|S| < ~1 given the 0.02-scaled weights).

Partition alignment: compute engines are lane-locked (PSUM partition p ->
SBUF partition p), so per-head feature layouts alternate by head parity
(even heads [qk;conv], odd heads [conv;qk]) making every PSUM->SBUF copy
partition-aligned; the few genuinely shifting copies (odd-head attention
outputs, denominator rows) go through DMA, which can move partitions freely.
All permutation bookkeeping is done host-side in numpy when preparing
per-core inputs.
"""

import os
import sys

sys.path.insert(0, "/opt/trn_rl_repo")

import numpy as np

# ---------------------------------------------------------------- constants
B, N, DIM, HEAD, VQE_K = 4, 2048, 512, 8, 3
Dh = DIM // HEAD            # 64
HPC = HEAD // 2             # heads per core (8 cores = 4 batch * 2 groups)
P = 128
FB = 512                    # free-dim block (one fp32 PSUM bank)
NQB = N // FB               # 4
NKB = N // P                # 16
SCALE_Q = Dh ** -0.5 * 0.5  # folds the 0.5 score scale into the q/conv1 side

# which matmul groups run in float32r (1 cyc/row) vs float32 (4 cyc/row)
_DEFAULT_CFG = {"qkv": "f32", "attn": "f32", "proj": "f32"}

_CACHE = {}


# ---------------------------------------------------------------- host prep
def _host_prep(core, inp):
    """Build the per-core input arrays (sharding + layout permutations)."""
    b, g = core // 2, core % 2
    f32 = np.float32
    x, m, s = inp["x"], inp["m"], inp["s"]
    qkv_w, qkv_b = inp["qkv_w"], inp["qkv_b"]
    proj_w = inp["proj_w"]
    p1w = inp["pe1_w"].reshape(HEAD, VQE_K)
    p2w = inp["pe2_w"].reshape(HEAD, VQE_K)
    pe1_b, pe2_b = inp["pe1_b"], inp["pe2_b"]

    d = {}
    d["xt"] = np.ascontiguousarray(x[b].T, dtype=f32)  # [512, 2048]

    # m/s transposed, tile t rows = [head(2t+1) feats ; head(2t) feats]
    mt = np.empty((256, N), f32)
    st = np.empty((256, N), f32)
    mcw = np.zeros((128, 8), f32)
    scw = np.zeros((128, 8), f32)
    for t in range(2):
        h_lo, h_hi = g * 4 + 2 * t + 1, g * 4 + 2 * t
        mt[t * 128:t * 128 + 64] = m[b][:, h_lo * 64:(h_lo + 1) * 64].T
        mt[t * 128 + 64:t * 128 + 128] = m[b][:, h_hi * 64:(h_hi + 1) * 64].T
        st[t * 128:t * 128 + 64] = s[b][:, h_lo * 64:(h_lo + 1) * 64].T
        st[t * 128 + 64:t * 128 + 128] = s[b][:, h_hi * 64:(h_hi + 1) * 64].T
        for p in range(128):
            h = g * 4 + 2 * t + (1 if p < 64 else 0)
            mcw[p, 4 * t:4 * t + 3] = p1w[h] * SCALE_Q
            scw[p, 4 * t:4 * t + 3] = p2w[h]
            mcw[p, 4 * t + 3] = pe1_b[h] * SCALE_Q
            scw[p, 4 * t + 3] = pe2_b[h]
    d["mt"], d["st"], d["mcw"], d["scw"] = mt, st, mcw, scw

    # q/k projection weights: chunk ch=(t, q|k) = [even-head rows; odd-head rows]
    wqk_f = np.empty((512, DIM), f32)
    qkb = np.zeros((128, 4), f32)
    for t in range(2):
        for j in range(2):  # 0=q, 1=k
            ch = 2 * t + j
            h_e, h_o = g * 4 + 2 * t, g * 4 + 2 * t + 1
            base = j * DIM
            wqk_f[ch * 128:ch * 128 + 64] = qkv_w[base + h_e * 64:base + (h_e + 1) * 64]
            wqk_f[ch * 128 + 64:(ch + 1) * 128] = qkv_w[base + h_o * 64:base + (h_o + 1) * 64]
            qkb[0:64, ch] = qkv_b[base + h_e * 64:base + (h_e + 1) * 64]
            qkb[64:128, ch] = qkv_b[base + h_o * 64:base + (h_o + 1) * 64]
            if j == 0:
                wqk_f[ch * 128:(ch + 1) * 128] *= SCALE_Q
                qkb[:, ch] *= SCALE_Q
    d["wqk"] = np.ascontiguousarray(wqk_f.T)  # [c=512, f=512]
    d["qkb"] = qkb

    d["wv"] = np.ascontiguousarray(
        qkv_w[2 * DIM + g * 256:2 * DIM + (g + 1) * 256].T, dtype=f32)  # [512, 256]

    # proj rows / v-bias / one-hot broadcast matrix in aT partition order:
    # aT tile t partition p -> head 2t+(p>=64), d=p%64
    pjt = np.empty((256, DIM), f32)
    vbv = np.empty((256,), f32)
    for t in range(2):
        for p in range(128):
            h_l = 2 * t + (1 if p >= 64 else 0)
            h = g * 4 + h_l
            pjt[t * 128 + p] = proj_w[:, h * 64 + (p % 64)]
            vbv[t * 128 + p] = qkv_b[2 * DIM + h * 64 + (p % 64)]
    d["pjt"] = pjt
    d["vbv"] = np.ascontiguousarray(vbv.reshape(2, 128).T)  # [128, 2]
    return d


# ------------------------------------------------------------- device build
def _emit(tc, nc, io, cfg):
    from contextlib import ExitStack

    from concourse import mybir

    dt = mybir.dt
    f32 = dt.float32
    AF = mybir.ActivationFunctionType
    ALU = mybir.AluOpType

    # float32r tiles must be *produced* as float32r (the BIR verifier
    # requires producer-side rounding), so the dtype is set on the tiles
    # themselves rather than bitcast at the matmul call sites.
    def _dt(v):
        return {"f32": f32, "f32r": dt.float32r, "bf16": dt.bfloat16}[v]

    dt_qkv = _dt(cfg["qkv"])
    dt_attn = _dt(cfg["attn"])
    dt_proj = _dt(cfg["proj"])
    # exp granularity: one ACT op per FBS-wide stripe (matmuls within are
    # still 512-wide: a matmul output cannot cross a PSUM bank)
    FBS = 1024 if dt_attn == dt.bfloat16 else 512
    FBQ = 1024 if dt_qkv == dt.bfloat16 else 512
    NIH = FBS // FB

    with ExitStack() as ctx:
        persist = ctx.enter_context(tc.tile_pool(name="persist", bufs=1))

        # ---- persistent weight / activation tiles
        wqk_sb, wv_sb, xt_sb = [], [], []
        QP, KP, v_sb, aT, pjt_sb, bcsb = [], [], [], [], [], []
        for c in range(4):
            w = persist.tile([128, 512], dt_qkv, name=f"wqk{c}", tag=f"wqk{c}")
            nc.sync.dma_start(w[:], io["wqk"][c * 128:(c + 1) * 128, :])
            wqk_sb.append(w)
        mcw_sb = persist.tile([128, 8], f32, name="mcw", tag="mcw")
        nc.gpsimd.dma_start(mcw_sb[:], io["mcw"][:, :])
        scw_sb = persist.tile([128, 8], f32, name="scw", tag="scw")
        nc.gpsimd.dma_start(scw_sb[:], io["scw"][:, :])
        qkb_sb = persist.tile([128, 4], f32, name="qkb", tag="qkb")
        nc.sync.dma_start(qkb_sb[:], io["qkb"][:, :])

        for h in range(HPC):
            QP.append(persist.tile([128, N], dt_attn, name=f"QP{h}", tag=f"QP{h}"))
            KP.append(persist.tile([128, N], dt_attn, name=f"KP{h}", tag=f"KP{h}"))
        # per-head V block is [v(64) | ones | zero-pad] = 66 columns: matmul
        # operands need 4-byte-aligned offsets, so the block width must be
        # even for 2-byte dtypes (66*h*2 is always 4-aligned)
        for blk in range(NKB):
            v_sb.append(persist.tile([128, HPC * 66], dt_attn, name=f"vsb{blk}",
                                     tag=f"vsb{blk}"))
        for t in range(2):
            aT.append(persist.tile([128, N], dt_proj, name=f"aT{t}", tag=f"aT{t}"))
            bcsb.append(persist.tile([128, N], f32, name=f"bcsb{t}",
                                     tag=f"bcsb{t}"))
        # softmax denominators, DMA-reshaped onto all 128 partitions so the
        # reciprocal runs 32x wider than a [4, N] row layout would allow
        denR = persist.tile([128, N // 32], f32, name="denR", tag="denR")

        # ---- depthwise convs (DVE; emitted first so they overlap the qkv
        # matmuls — conv inputs stream on the gpsimd DMA queue)
        with tc.tile_pool(name="conv", bufs=2) as convp:
            for src, wv_, dst in (("mt", mcw_sb, QP), ("st", scw_sb, KP)):
                for t in range(2):
                    xin = convp.tile([128, N], f32, name=f"ci_{src}{t}", tag="cin")
                    nc.gpsimd.dma_start(xin[:], io[src][t * 128:(t + 1) * 128, :])
                    y = convp.tile([128, N], f32, name=f"cy_{src}{t}", tag="cy",
                                   bufs=1)
                    w0, w1, w2, cb = (wv_[:, 4 * t + k:4 * t + k + 1]
                                      for k in range(4))
                    nc.vector.tensor_scalar(y[:], xin[:], w1, cb,
                                            ALU.mult, ALU.add)
                    nc.vector.scalar_tensor_tensor(
                        y[:, 1:], xin[:, :N - 1], w0, y[:, 1:],
                        ALU.mult, ALU.add)
                    nc.vector.scalar_tensor_tensor(
                        y[:, :N - 1], xin[:, 1:], w2, y[:, :N - 1],
                        ALU.mult, ALU.add)
                    nc.vector.tensor_copy(dst[2 * t + 1][0:64, :], y[0:64, :])
                    nc.vector.tensor_copy(dst[2 * t][64:128, :], y[64:128, :])

        # ---- qkv projections (x^T resident only here)
        with tc.tile_pool(name="xtp", bufs=1) as xtp:
            for c in range(4):
                xt = xtp.tile([128, N], dt_qkv, name=f"xt{c}", tag=f"xt{c}")
                nc.sync.dma_start(xt[:], io["xt"][c * 128:(c + 1) * 128, :])
                xt_sb.append(xt)
            for c in range(4):
                w = persist.tile([128, 256], dt_qkv, name=f"wv{c}", tag=f"wv{c}")
                nc.sync.dma_start(w[:], io["wv"][c * 128:(c + 1) * 128, :])
                wv_sb.append(w)
            vbv_sb = persist.tile([128, 2], f32, name="vbv", tag="vbv")
            nc.sync.dma_start(vbv_sb[:], io["vbv"][:, :])
            for f in range(2):
                w = persist.tile([128, 512], dt_proj, name=f"pjt{f}", tag=f"pjt{f}")
                nc.sync.dma_start(w[:], io["pjt"][f * 128:(f + 1) * 128, :])
                pjt_sb.append(w)

            with tc.tile_pool(name="ps_qkv", bufs=1, space="PSUM") as ps_qkp:
                for t in range(2):
                    for j in range(2):
                        ch = 2 * t + j
                        dst = QP if j == 0 else KP
                        for qb in range(N // FBQ):
                            qs = slice(qb * FBQ, (qb + 1) * FBQ)
                            ps = ps_qkp.tile([128, FBQ], f32, name="psqk",
                                             tag="psqk", bufs=3)
                            for ih in range(FBQ // FB):
                                hqs = slice(qb * FBQ + ih * FB,
                                            qb * FBQ + (ih + 1) * FB)
                                for c in range(4):
                                    nc.tensor.matmul(
                                        ps[:, ih * FB:(ih + 1) * FB],
                                        wqk_sb[c][:, ch * 128:(ch + 1) * 128],
                                        xt_sb[c][:, hqs],
                                        start=(c == 0), stop=(c == 3))
                            nc.vector.tensor_scalar_add(
                                dst[2 * t][0:64, qs], ps[0:64, :],
                                qkb_sb[0:64, ch:ch + 1])
                            nc.vector.tensor_scalar_add(
                                dst[2 * t + 1][64:128, qs], ps[64:128, :],
                                qkb_sb[64:128, ch:ch + 1])
                for blk in range(NKB):
                    bs = slice(blk * 128, (blk + 1) * 128)
                    ps = ps_qkp.tile([128, 256], f32, name="psv", tag="psv",
                                     bufs=2)
                    for c in range(4):
                        nc.tensor.matmul(ps[:], xt_sb[c][:, bs],
                                         wv_sb[c][:],
                                         start=(c == 0), stop=(c == 3))
                    v3 = v_sb[blk].rearrange("p (h f) -> p h f", h=HPC)
                    nc.vector.tensor_copy(v3[:, :, 0:64],
                                          ps.rearrange("p (h f) -> p h f", h=HPC))
                    # memset lacks float32r support; write the ones/pad columns
                    # through an f32 view (identical bit pattern)
                    ones_ap, pad_ap = v3[:, :, 64:65], v3[:, :, 65:66]
                    if dt_attn == dt.float32r:
                        ones_ap = ones_ap.bitcast(f32)
                        pad_ap = pad_ap.bitcast(f32)
                    nc.vector.memset(ones_ap, 1.0)
                    nc.vector.memset(pad_ap, 0.0)

        # ---- attention (fused score matmul + exp + attn@[V|1|0])
        # per-(head, stripe) PSUM output tiles double-buffer so the next
        # group's accumulation starts while the previous one is copied out
        with tc.tile_pool(name="ps_s", bufs=2, space="PSUM") as ps_sp, \
                tc.tile_pool(name="ps_o", bufs=2, space="PSUM") as ps_op, \
                tc.tile_pool(name="esbp", bufs=2) as esbp, \
                tc.tile_pool(name="stg", bufs=2) as stgp:
            for h in range(HPC):
                t, odd = h // 2, h % 2
                vcols = slice(h * 66, (h + 1) * 66)
                for q2 in range(N // FBS):
                    qbase = q2 * FBS
                    cs = slice(qbase, qbase + FBS)
                    o_ps = ps_op.tile([66, FBS], f32, name=f"ops{h}_{q2}",
                                      tag="ops")
                    for nk in range(NKB):
                        ks = slice(nk * 128, (nk + 1) * 128)
                        s_ps = ps_sp.tile([128, FBS], f32, name="sps",
                                          tag="sps")
                        for ih in range(NIH):
                            hqs = slice(qbase + ih * FB, qbase + (ih + 1) * FB)
                            nc.tensor.matmul(s_ps[:, ih * FB:(ih + 1) * FB],
                                             KP[h][:, ks], QP[h][:, hqs],
                                             start=True, stop=True)
                        e_sb = esbp.tile([128, FBS], dt_attn, name="esb",
                                         tag="esb")
                        nc.scalar.activation(e_sb[:], s_ps[:], AF.Exp)
                        for ih in range(NIH):
                            nc.tensor.matmul(
                                o_ps[:, ih * FB:(ih + 1) * FB],
                                v_sb[nk][:, vcols],
                                e_sb[:, ih * FB:(ih + 1) * FB],
                                start=(nk == 0), stop=(nk == NKB - 1))
                    # lane-locked engines cannot shift partitions and DMA
                    # cannot read PSUM, so shifting copies stage through SBUF
                    stgd = stgp.tile([65, FBS], f32, name=f"sd{h}_{q2}",
                                     tag="stgd")
                    if odd:
                        stg = stgp.tile([64, FBS], dt_proj, name=f"sg{h}_{q2}",
                                        tag="stg")
                        nc.vector.tensor_copy(stg[:], o_ps[0:64, :])
                        nc.sync.dma_start(aT[t][64:128, cs], stg[:])
                    else:
                        nc.vector.tensor_copy(aT[t][0:64, cs], o_ps[0:64, :])
                    nc.vector.tensor_copy(stgd[64:65, :], o_ps[64:65, :])
                    # denominator stripe -> denR rows (DMA-reshaped, linear)
                    r0 = h * 32 + q2 * (FBS // 64)
                    nc.sync.dma_start(denR[r0:r0 + FBS // 64, :],
                                      stgd[64:65, :])
                if odd:
                    # both heads of aT[t] done: reciprocal + DMA-replicated
                    # broadcast + normalize, overlapped with later heads
                    nc.vector.reciprocal(denR[t * 64:(t + 1) * 64, :],
                                         denR[t * 64:(t + 1) * 64, :])
                    nc.sync.dma_start(io["drec"][2 * t:2 * t + 2, :],
                                      denR[t * 64:(t + 1) * 64, :])
                    for par in range(2):
                        nc.sync.dma_start(
                            bcsb[t][par * 64:(par + 1) * 64, :],
                            io["drec"][2 * t + par:2 * t + par + 1,
                                       :].broadcast_to([64, N]))
                    nc.vector.tensor_mul(aT[t][:], aT[t][:], bcsb[t][:])
                    nc.vector.tensor_scalar_add(aT[t][:], aT[t][:],
                                                vbv_sb[:, t:t + 1])

        # ---- output projection (partial over this core's heads)
        with tc.tile_pool(name="ps_pj", bufs=3, space="PSUM") as ps_pjp, \
                tc.tile_pool(name="osbp", bufs=3) as osbp:
            for blk in range(NKB):
                bs = slice(blk * 128, (blk + 1) * 128)
                pj = ps_pjp.tile([128, FB], f32, name="pj", tag="pj")
                for f in range(2):
                    nc.tensor.matmul(pj[:], aT[f][:, bs],
                                     pjt_sb[f][:],
                                     start=(f == 0), stop=(f == 1))
                ob = osbp.tile([128, FB], f32, name="ob", tag="ob")
                nc.vector.tensor_copy(ob[:], pj[:])
                nc.gpsimd.dma_start(io["out"][bs, :], ob[:])


def _build(cfg_key):
    from concourse import bacc, mybir, tile

    cfg = dict(cfg_key)
    dt = mybir.dt
    nc = bacc.Bacc("TRN2", target_bir_lowering=False, debug=False,
                   num_devices=8)
    _d = {"f32": dt.float32, "f32r": dt.float32r, "bf16": dt.bfloat16}
    dt_qkv = _d[cfg["qkv"]]
    dt_proj = _d[cfg["proj"]]
    shapes = {
        "xt": ([DIM, N], dt_qkv), "mt": ([256, N], dt.float32),
        "st": ([256, N], dt.float32),
        "wqk": ([DIM, 512], dt_qkv), "wv": ([DIM, 256], dt_qkv),
        "pjt": ([256, DIM], dt_proj),
        "mcw": ([128, 8], dt.float32),
        "scw": ([128, 8], dt.float32),
        "qkb": ([128, 4], dt.float32), "vbv": ([128, 2], dt.float32),
    }
    io = {}
    for name, (shape, dtt) in shapes.items():
        io[name] = nc.dram_tensor(name, shape, dtt,
                                  kind="ExternalInput").ap()
    io["out"] = nc.dram_tensor("out", [N, DIM], dt.float32,
                               kind="ExternalOutput").ap()
    # internal DRAM bounce for the denominator broadcast (DMA cannot
    # replicate from an SBUF source, but a DRAM source AP is linear and
    # supports a zero-step leading dim)
    io["drec"] = nc.dram_tensor("drec", [4, N], dt.float32).ap()
    with tile.TileContext(nc) as tc:
        _emit(tc, nc, io, cfg)
    nc.compile()
    return nc


def _get_program(cfg):
    key = tuple(sorted(cfg.items()))
    if key not in _CACHE:
        _CACHE[key] = _build(key)
    return _CACHE[key]


# ------------------------------------------------------------------ wrapper
def kernel(_cfg=None, _want_results=False, **inputs):
    from concourse.bass_utils import run_bass_kernel_spmd

    cfg = dict(_DEFAULT_CFG)
    if _cfg:
        cfg.update(_cfg)
    env_cfg = os.environ.get("BASSKERN_CFG")
    if env_cfg:  # e.g. "attn=f32r,qkv=f32r"
        for kv in env_cfg.split(","):
            k, v = kv.split("=")
            cfg[k] = v

    inputs = {k: np.asarray(v, dtype=np.float32) for k, v in inputs.items()}
    nc = _get_program(cfg)
    in_maps = [_host_prep(core, inputs) for core in range(8)]
    # bf16 configs declare the corresponding DRAM tensors as bfloat16
    conv_keys = []
    if cfg["qkv"] == "bf16":
        conv_keys += ["xt", "wqk", "wv"]
    if cfg["proj"] == "bf16":
        conv_keys += ["pjt"]
    if conv_keys:
        import ml_dtypes
        for im in in_maps:
            for k in conv_keys:
                im[k] = im[k].astype(ml_dtypes.bfloat16)
    res = run_bass_kernel_spmd(nc, in_maps, list(range(8)))

    out = np.empty((B, N, DIM), np.float32)
    pb = inputs["proj_b"]
    for b in range(B):
        out[b] = res.results[2 * b]["out"] + res.results[2 * b + 1]["out"] + pb
    if _want_results:
        return out, res
    return out

